# revision 27
# baseline (speedup 1.0000x reference)
"""Trainium2 Bass kernel for nn_BinaryNetFCBlock.

Computes  y = BN(sign(x) @ sign(k))  where
  sign(v) = +1 if v >= 0 else -1            (larq ste_sign forward)
  BN(y)   = (y - moving_mean) * rsqrt(moving_var + 1e-3) + beta

Full shapes: x [8192, 4096] f32, k [4096, 4096] f32, BN params [4096].

Sharding: 2D, 4 batch-groups x 2 n-groups across 8 NeuronCores.
Each core: x-shard [2048, 4096] @ k-shard [4096, 2048] -> y-shard
[2048, 2048].  This balances per-core HBM traffic (33.6 MB x + 33.6 MB
k vs 16.8 + 67.1 for pure data-parallel).

Host-side layout prep (sharding = layout choice, all math on device):
  x-shard is pre-transposed + pair-interleaved on the host into
    xt [BL/XGB, 128, NJJ, 2, XGB] f32,  d = jj*256 + 2*p + ko
  so the contraction dim lands on partitions with fully contiguous
  per-partition DMA reads, eliminating the on-device sign->DRAM->
  xbar-transpose round trip entirely (the old serial 200us prefix).
  k-shard likewise into kt [NL/KGN, 128, NJJ, 2, KGN] f32.
  BN param vectors are reshaped to [128, NT] (p-major).

Operands are marshaled host-side to bf16 by BIT-TRUNCATING f32 (drop
the low mantissa halfword): the sign/exponent bits are untouched, so
the device-side sign decision (v >= 0) is bit-identical to f32 while
halving load bytes (supply 33.6 MB vs PE ~224 us -> full slack).

Per-core device pipeline.  Every engine queue is FIFO, so each queue
carries only one "pace class" (load-paced vs PE-paced) in temporal
order:
  gpsimd ring : all operand group loads, one queue, in exactly the
                order the tile wavefront needs them (load-paced)
  scalar ring : BN param loads at t=0, then epilogue stores (PE-paced)
  DVE         : param math, then all sign ops (load-paced)
                sign via (v >= 0) - 0.5 -> +-0.5 exact in fp8
  PE          : fp8 DoubleRow matmuls, lhsT = kq slice [128, 2, 128],
                rhs = xq slice [128, 2, 256]; PSUM [128, 256]
                accumulates yT over K=4096 (16 DR steps).  FD=256
                streams at full rate (109 ns/MM): the DR LDWEIGHTS
                loads only 128 columns and hides in the PE's reorder
                window, so the fine tile grid is free and the supply
                wavefront smooth.  Measured ~98% of the 157 TF/s fp8
                peak during the MM phase.
  ACT         : epilogue Identity(psum * s[n] + t[n]) -> bf16, store
                follows its epilogue on the same scalar queue
                s = 4*rsqrt(var+eps) (4 compensates +-0.5 * +-0.5),
                t = beta - mean*rsqrt(var+eps)
All loads+signs are emitted up front (pool bufs gate staging); tiles
(nt, b-chunk) are emitted in a greedy wavefront order against a
DMA-arrival model (fixed-point between tile order and group order) so
the PE starts ~20us in and never starves.
Output: yT [2048, 2048] bf16 per core; host transposes/concats/f32.
"""

import sys

for _p in ("/opt/trn_rl_repo",):
    if _p not in sys.path:
        sys.path.append(_p)

import contextlib

import numpy as np

import concourse.bass as bass
import concourse.mybir as mybir
import concourse.tile as tile
from concourse import bacc

F32 = mybir.dt.float32
BF16 = mybir.dt.bfloat16
FP8 = mybir.dt.float8e4
AF = mybir.ActivationFunctionType
ALU = mybir.AluOpType
DR = mybir.MatmulPerfMode.DoubleRow

BN_EPS = 1e-3
P = 128


def emit_kernel(tc, outs, ins, cfg):
    nc = tc.nc
    BL, NL, D = cfg["BL"], cfg["NL"], cfg["D"]
    XGB, KGN, BC = cfg["XGB"], cfg["KGN"], cfg["BC"]
    NJJ = D // (2 * P)        # 16 DR steps over K
    NT = NL // P              # 16 output n-tiles
    NBC = BL // BC            # 4 b chunks
    NXG = BL // XGB           # 32 x load groups
    NKG = NL // KGN           # 32 k load groups
    KPT = P // KGN            # k groups per n-tile
    XPB = BC // XGB           # x groups per b chunk

    xt_ap = ins["xt"]
    kt_ap = ins["kt"]
    var_ap = ins["var_t"]
    mean_ap = ins["mean_t"]
    beta_ap = ins["beta_t"]
    yT_ap = outs["outT"]

    with contextlib.ExitStack() as ctx:
        pool = lambda name, bufs, **kw: ctx.enter_context(
            tc.tile_pool(name=name, bufs=bufs, **kw)
        )
        stp = pool("stp", 1)
        xqp = pool("xq", 1)
        kqp = pool("kq", 1)
        xload = pool("xload", cfg.get("xload_bufs", 4))
        kload = pool("kload", cfg.get("kload_bufs", 4))
        psum = pool("psum", cfg.get("psum_bufs", 8), space="PSUM")
        osb = pool("osb", cfg.get("osb_bufs", 4))

        # ---- BN parameter prep (host supplies [128, NT] p-major views).
        # Param DMAs ride the scalar ring (idle until epilogues) so the
        # two load rings start streaming operand bytes at t=0.
        pv = stp.tile([P, 3 * NT], F32)
        nc.scalar.dma_start(pv[:, 0:NT], var_ap)
        nc.scalar.dma_start(pv[:, NT : 2 * NT], mean_ap)
        nc.scalar.dma_start(pv[:, 2 * NT : 3 * NT], beta_ap)
        eps_t = stp.tile([P, 1], F32)
        nc.vector.memset(eps_t[:], BN_EPS)
        sq = stp.tile([P, NT], F32)
        nc.scalar.activation(sq[:], pv[:, 0:NT], AF.Sqrt, bias=eps_t[:])
        inv = stp.tile([P, NT], F32)
        nc.vector.reciprocal(inv[:], sq[:])
        ms = stp.tile([P, NT], F32)
        nc.vector.tensor_mul(ms[:], pv[:, NT : 2 * NT], inv[:])
        t_sb = stp.tile([P, NT], F32)
        nc.vector.tensor_sub(t_sb[:], pv[:, 2 * NT : 3 * NT], ms[:])
        s_sb = stp.tile([P, NT], F32)
        # both operands encoded +-0.5 -> products x4 -> compensate with 4x
        nc.vector.tensor_scalar(s_sb[:], inv[:], 4.0, None, op0=ALU.mult)

        # ---- resident operand tiles, produced group-wise as DMAs land
        xq = xqp.tile([P, NJJ, 2, BL], FP8)
        kq = kqp.tile([P, NJJ, 2, NL], FP8)

        def emit_group(kind, g):
            if kind == "x":
                src, dst, pool_, w = xt_ap, xq, xload, XGB
            else:
                src, dst, pool_, w = kt_ap, kq, kload, KGN
            ld = pool_.tile([P, NJJ, 2, w], BF16)
            nc.gpsimd.dma_start(ld[:], src[g])
            nc.vector.tensor_scalar(
                dst[:, :, :, g * w : (g + 1) * w],
                ld[:],
                0.0,
                0.5,
                op0=ALU.is_ge,
                op1=ALU.subtract,
            )

        # ---- schedule: fixed-point between tile order and DMA arrival.
        # All groups ride one SWDGE queue in exactly the order tiles
        # first need them.  Tile ready = latest needed group + sign.
        assert XGB == KGN
        GMB = (P * NJJ * 2 * XGB * 2) / 1e6   # bf16 groups
        RATE = 0.280   # MB/us aggregate (conservative; calibrated on HW)
        RAMP = 6.0     # us until first bytes flow
        SIGNL = 2.0    # us sign latency

        # Tile list: (order_key, nt, b0, bw).  FD=256 tiles run at full
        # streaming rate (DoubleRow LDWEIGHTS loads only 128 columns and
        # hides in the reorder window), so the fine grid costs ~nothing
        # and gives a much smoother supply wavefront.
        def tile_defs():
            return [(nt, bc * BC, BC) for nt in range(NT) for bc in range(BL // BC)]

        def xgroups(b0, bw):
            return range(b0 // XGB, (b0 + bw - 1) // XGB + 1)

        tiles = [
            (max(nt, (b0 // BC) * 2), nt, b0, bw) for nt, b0, bw in tile_defs()
        ]
        tiles.sort()
        for _ in range(3):
            gorder = []
            seen = set()
            for _, nt, b0, bw in tiles:
                for g in xgroups(b0, bw):
                    if ("x", g) not in seen:
                        seen.add(("x", g))
                        gorder.append(("x", g))
                for j in range(KPT):
                    if ("k", nt * KPT + j) not in seen:
                        seen.add(("k", nt * KPT + j))
                        gorder.append(("k", nt * KPT + j))
            tx, tk = [0.0] * NXG, [0.0] * NKG
            for i, (kind, g) in enumerate(gorder):
                t = RAMP + (i + 1) * GMB / RATE
                (tx if kind == "x" else tk)[g] = t
            tiles = []
            for nt, b0, bw in tile_defs():
                rx = max(tx[g] for g in xgroups(b0, bw))
                rk = max(tk[nt * KPT + j] for j in range(KPT))
                tiles.append((max(rx, rk) + SIGNL, nt, b0, bw))
            tiles.sort()

        for kind, g in gorder:
            emit_group(kind, g)

        for _, nt, b0, bw in tiles:
            ps = psum.tile([P, bw], F32, tag=f"ps{bw}", bufs=8)
            for jj in range(NJJ):
                nc.tensor.matmul(
                    ps[:],
                    kq[:, jj, :, nt * P : (nt + 1) * P],
                    xq[:, jj, :, b0 : b0 + bw],
                    start=(jj == 0),
                    stop=(jj == NJJ - 1),
                    perf_mode=DR,
                )
            ob = osb.tile([P, bw], BF16, tag=f"ob{bw}", bufs=4)
            nc.scalar.activation(
                ob[:],
                ps[:],
                AF.Identity,
                bias=t_sb[:, nt : nt + 1],
                scale=s_sb[:, nt : nt + 1],
            )
            nc.scalar.dma_start(yT_ap[nt * P : (nt + 1) * P, b0 : b0 + bw], ob[:])


def build_nc(cfg):
    """Build + compile the Bacc module for one core (SPMD: same for all)."""
    BL, NL, D = cfg["BL"], cfg["NL"], cfg["D"]
    XGB, KGN = cfg["XGB"], cfg["KGN"]
    NJJ = D // (2 * P)
    NT = NL // P
    nc = bacc.Bacc(
        "TRN2", target_bir_lowering=False, debug=False, enable_asserts=True
    )
    ins = {
        "xt": nc.dram_tensor(
            "xt", [BL // XGB, P, NJJ, 2, XGB], BF16, kind="ExternalInput"
        ).ap(),
        "kt": nc.dram_tensor(
            "kt", [NL // KGN, P, NJJ, 2, KGN], BF16, kind="ExternalInput"
        ).ap(),
        "var_t": nc.dram_tensor("var_t", [P, NT], F32, kind="ExternalInput").ap(),
        "mean_t": nc.dram_tensor("mean_t", [P, NT], F32, kind="ExternalInput").ap(),
        "beta_t": nc.dram_tensor("beta_t", [P, NT], F32, kind="ExternalInput").ap(),
    }
    outs = {
        "outT": nc.dram_tensor("outT", [NL, BL], BF16, kind="ExternalOutput").ap(),
    }
    with tile.TileContext(nc) as tc:
        emit_kernel(tc, outs, ins, cfg)
    nc.compile()
    return nc


FULL_CFG = dict(BL=2048, NL=2048, D=4096, XGB=128, KGN=128, BC=256)
SB, SN = 4, 2
N_CORES = SB * SN

_cached = {}


def _get_nc(key, cfg):
    if key not in _cached:
        _cached[key] = build_nc(cfg)
    return _cached[key]


def kernel(input_tensor, kernel, beta, moving_mean, moving_var, trace=False):
    from concourse.bass_utils import run_bass_kernel_spmd

    B, D = input_tensor.shape
    N = kernel.shape[1]
    BL, NL = B // SB, N // SN
    cfg = dict(FULL_CFG, BL=BL, NL=NL, D=D)
    nc = _get_nc(("full", BL, NL, D), cfg)

    NJJ = D // (2 * P)
    XGB, KGN = cfg["XGB"], cfg["KGN"]

    import ml_dtypes

    def to_bf16(a):
        # Bit-truncate f32 -> bf16: keeps the exact sign/exponent bits,
        # so the device-side sign decision (v >= 0) is identical to the
        # f32 one for every input value.
        a = np.ascontiguousarray(a, dtype=np.float32)
        return (
            (a.view(np.uint32) >> np.uint32(16))
            .astype(np.uint16)
            .view(ml_dtypes.bfloat16)
        )

    # Host-side sharding + layout packing (pure data movement).
    xt_arrs = []
    for cb in range(SB):
        xs = np.asarray(input_tensor[cb * BL : (cb + 1) * BL, :], dtype=np.float32)
        a = xs.T.reshape(NJJ, P, 2, BL // XGB, XGB).transpose(3, 1, 0, 2, 4)
        xt_arrs.append(to_bf16(a))
    kt_arrs = []
    pv_arrs = []
    for cn in range(SN):
        ks = np.asarray(kernel[:, cn * NL : (cn + 1) * NL], dtype=np.float32)
        a = ks.reshape(NJJ, P, 2, NL // KGN, KGN).transpose(3, 1, 0, 2, 4)
        kt_arrs.append(to_bf16(a))
        pv_arrs.append(
            tuple(
                np.ascontiguousarray(
                    np.asarray(v[cn * NL : (cn + 1) * NL], dtype=np.float32)
                    .reshape(-1, P)
                    .T
                )
                for v in (moving_var, moving_mean, beta)
            )
        )

    in_maps = []
    for c in range(N_CORES):
        cb, cn = c // SN, c % SN
        vt, mt, bt = pv_arrs[cn]
        in_maps.append(
            {
                "xt": xt_arrs[cb],
                "kt": kt_arrs[cn],
                "var_t": vt,
                "mean_t": mt,
                "beta_t": bt,
            }
        )
    kw = {}
    if trace:
        kw["trace_cores"] = list(range(N_CORES))
    res = run_bass_kernel_spmd(
        nc, in_maps, core_ids=list(range(N_CORES)), trace=trace, **kw
    )
    out = np.empty((B, N), dtype=np.float32)
    for c in range(N_CORES):
        cb, cn = c // SN, c % SN
        yT = np.asarray(res.results[c]["outT"], dtype=np.float32)
        out[cb * BL : (cb + 1) * BL, cn * NL : (cn + 1) * NL] = yT.T
    if trace:
        return out, res
    return out
